# revision 11
# baseline (speedup 1.0000x reference)
"""Multi-head self-attention (B=4, T=2048, D=1024, H=16) on 8 trn2 cores — v4.

Sharding: core = b * 2 + g (b = batch, g = head-group of 8 heads). Host sums
the two head-group partials per batch.

Structure (single software pipeline):
  - QKV projection, attention and out-projection groups interleaved so
    ACT/DVE (the exp engines) start immediately and the PE fills exp-wait
    slack with projection work.
  - Q/K projections run fp8(e4m3) DoubleRow (K_eff=256 per matmul): half the
    matmul instructions. V / PV / out-proj stay bf16.
  - PSUM: pool PA = 4 x [128,512] (scores halves, projection group halves),
    pool PC = 4 x [65,512] (PV accumulators, 2 blocks in flight).
  - Scores pair h0/h64 are adjacent row-tiled matmuls (concurrent on PE);
    exp runs per half, split across ACT and DVE per (si, half).
  - Normalization: denominator row pulls on ACT, reciprocal bit-trick +
    one Newton step on DVE ([1,1024]), partition broadcasts on gpsimd.
  - Inputs spread over 4 DMA queues (sync/scalar/gpsimd/vector), fp8 x
    first (halves) so the first projection groups start ~6us in.
"""

import numpy as np
import ml_dtypes
import concourse.bass as bass
import concourse.bacc as bacc
import concourse.mybir as mybir
import concourse.tile as tile
from concourse.bass_utils import run_bass_kernel_spmd

B, T, D = 4, 2048, 1024
H, DK = 16, 64
G = 2
HPG = H // G          # 8 heads per core
HD = HPG * DK         # 512
NCORES = B * G
SCALE = 1.0 / float(np.sqrt(DK))

F32 = mybir.dt.float32
BF16 = mybir.dt.bfloat16
F8 = mybir.dt.float8e4
I32 = mybir.dt.int32
I16 = mybir.dt.int16
AT = mybir.AluOpType
Ident = mybir.ActivationFunctionType.Identity
Exp = mybir.ActivationFunctionType.Exp
DR = mybir.MatmulPerfMode.DoubleRow

NCC = D // 128        # 8 contraction chunks
NCP = NCC // 2        # 4 fp8 contraction pairs
NDT = HD // 128       # 4 head-pair tiles
NTT = T // 128        # 16 t-tiles
NSI = T // 128        # 16 s-tiles
NTB = T // 512        # 4 t-blocks
VW = DK + 1           # 65: v + ones column

# ---- exp constants (common output scale 2^-63/c2, cancels in softmax) ----
LOG2E = float(np.log2(np.e))
_ws = np.linspace(1, 2, 4001)
_c2, _c1, _c0 = np.polyfit(_ws, 2 ** (_ws - 1) / _ws, 2)
A16 = float((2 ** 7) * LOG2E * SCALE)
B16S = float((127 - 63) * 2 ** 7 + (2 ** 7) * np.log2(1.0 / _c2) - 7.25)
ACT_BIAS = float(-63 * np.log(2) - np.log(_c2))
C_RECIP = 2129850000.0
NR_SCALAR = 2.0 * (1.0 + 0.00066)
# si whose fused exp runs on ACT (others on DVE)
ACT_SI = {0, 2, 4, 6, 8, 10, 12, 14, 15}


def build_program():
    nc = bacc.Bacc("TRN2", target_bir_lowering=False, debug=False)

    xt = nc.dram_tensor("xt", [D, T], BF16, kind="ExternalInput").ap()
    xf8 = nc.dram_tensor("xf8", [128, NCP, 2, T], F8,
                         kind="ExternalInput").ap()
    wqf8 = nc.dram_tensor("wqf8", [128, NCP * 2 * HD], F8,
                          kind="ExternalInput").ap()
    wkf8 = nc.dram_tensor("wkf8", [128, NCP * 2 * HD], F8,
                          kind="ExternalInput").ap()
    wv = nc.dram_tensor("wv", [D, HD], BF16, kind="ExternalInput").ap()
    bq = nc.dram_tensor("bq", [HD, 1], F32, kind="ExternalInput").ap()
    bk = nc.dram_tensor("bk", [HD, 1], F32, kind="ExternalInput").ap()
    bv2 = nc.dram_tensor("bv2", [128, 2 * HD], F32, kind="ExternalInput").ap()
    wo = nc.dram_tensor("wo", [HD, D], BF16, kind="ExternalInput").ap()
    bo = nc.dram_tensor("bo", [128, D], F32, kind="ExternalInput").ap()
    y = nc.dram_tensor("y", [T, D], BF16, kind="ExternalOutput").ap()

    with tile.TileContext(nc) as tc:
        with tc.tile_pool(name="persist", bufs=1) as pp:
            qT = [pp.tile([128, T], BF16, name=f"qT{i}", tag=f"qT{i}")
                  for i in range(NDT)]
            kT = [pp.tile([128, T], BF16, name=f"kT{i}", tag=f"kT{i}")
                  for i in range(NDT)]
            ctx = [pp.tile([128, T], BF16, name=f"ctx{i}", tag=f"ctx{i}")
                   for i in range(NDT)]
            vv = pp.tile([128, NSI * HPG * VW], BF16, name="vv", tag="vv")
            xf8_sb = [pp.tile([128, 2, T], F8, name=f"xf8_{j}",
                              tag=f"xf8_{j}") for j in range(NCP)]
            xts = [pp.tile([128, T], BF16, name=f"xt{c}", tag=f"xt{c}")
                   for c in range(NCC)]
            wq_sb = [pp.tile([128, 2 * HD], F8, name=f"wq{j}", tag=f"wq{j}")
                     for j in range(NCP)]
            wk_sb = [pp.tile([128, 2 * HD], F8, name=f"wk{j}", tag=f"wk{j}")
                     for j in range(NCP)]
            wv_sb = [pp.tile([128, HD], BF16, name=f"wv{c}", tag=f"wv{c}")
                     for c in range(NCC)]
            wo_sb = [pp.tile([128, D], BF16, name=f"wo{c}", tag=f"wo{c}")
                     for c in range(NDT)]
            bq_sb = [pp.tile([128, 1], F32, name=f"bq{i}", tag=f"bq{i}")
                     for i in range(NDT)]
            bk_sb = [pp.tile([128, 1], F32, name=f"bk{i}", tag=f"bk{i}")
                     for i in range(NDT)]
            bv_sb = pp.tile([128, 2 * HD], F32, name="bv_sb", tag="bv_sb")
            bo_sb = pp.tile([128, D], F32, name="bo_sb", tag="bo_sb")
            actbias_sb = pp.tile([128, 1], F32, name="actbias", tag="actbias")
            nc.vector.memset(actbias_sb[:], ACT_BIAS)
            # ones columns of vv (never overwritten afterwards)
            v4 = vv[:].rearrange("p (s h e) -> p s h e", h=HPG, e=VW)
            nc.vector.memset(v4[:, :, :, DK:DK + 1], 1.0)

            # ---------------- input DMAs (3 queues) ----------------
            # sync: fp8 x low halves, xt 4-5, fp8 x high halves, wo
            nc.sync.dma_start(xf8_sb[0][:, :, 0:1024], xf8[:, 0, :, 0:1024])
            nc.sync.dma_start(xf8_sb[2][:, :, 0:1024], xf8[:, 2, :, 0:1024])
            for c in (4, 5):
                nc.sync.dma_start(xts[c][:], xt[c * 128:(c + 1) * 128, :])
            for j in range(NCP):
                nc.sync.dma_start(xf8_sb[j][:, :, 1024:2048],
                                  xf8[:, j, :, 1024:2048])
            for c in range(NDT):
                nc.sync.dma_start(wo_sb[c][:], wo[c * 128:(c + 1) * 128, :])
            # scalar: first fp8 weights + first biases, xf8 1/3 low, xt 6-7
            nc.scalar.dma_start(wk_sb[0][:], wkf8[:, 0:2 * HD])
            nc.scalar.dma_start(wq_sb[0][:], wqf8[:, 0:2 * HD])
            nc.scalar.dma_start(bk_sb[0][:], bk[0:128, :])
            nc.scalar.dma_start(bq_sb[0][:], bq[0:128, :])
            nc.scalar.dma_start(xf8_sb[1][:, :, 0:1024], xf8[:, 1, :, 0:1024])
            for j in range(1, NCP):
                nc.scalar.dma_start(wk_sb[j][:],
                                    wkf8[:, j * 2 * HD:(j + 1) * 2 * HD])
                nc.scalar.dma_start(wq_sb[j][:],
                                    wqf8[:, j * 2 * HD:(j + 1) * 2 * HD])
            nc.scalar.dma_start(xf8_sb[3][:, :, 0:1024], xf8[:, 3, :, 0:1024])
            for c in (6, 7):
                nc.scalar.dma_start(xts[c][:], xt[c * 128:(c + 1) * 128, :])
            for i in range(1, NDT):
                nc.scalar.dma_start(bq_sb[i][:], bq[i * 128:(i + 1) * 128, :])
                nc.scalar.dma_start(bk_sb[i][:], bk[i * 128:(i + 1) * 128, :])
            nc.scalar.dma_start(bv_sb[:], bv2[:])
            nc.scalar.dma_start(bo_sb[:], bo[:])
            # gpsimd: x chunks 0-3, wv, x 4-5 also arrive via sync
            for c in range(4):
                nc.gpsimd.dma_start(xts[c][:], xt[c * 128:(c + 1) * 128, :])
            for c in range(NCC):
                nc.gpsimd.dma_start(wv_sb[c][:], wv[c * 128:(c + 1) * 128, :])

            with tc.tile_pool(name="pa", bufs=2, space="PSUM") as pa, \
                 tc.tile_pool(name="pc", bufs=4, space="PSUM") as pc, \
                 tc.tile_pool(name="sb", bufs=1) as sb:

                # ---------------- group emitters ----------------
                def emit_qk_group(w_sb, b_sb, outT, dt, tcnp):
                    """qT/kT[dt][:, tcnp*1024 : +1024] via 8 fp8 DR matmuls."""
                    ps = pa.tile([128, 1024], F32, name="pa", tag="pa")
                    for j in range(NCP):
                        w3 = w_sb[j][:].rearrange("p (k m) -> p k m", k=2)
                        for hh in range(2):
                            t0 = tcnp * 1024 + hh * 512
                            nc.tensor.matmul(
                                ps[:, hh * 512:(hh + 1) * 512],
                                w3[:, :, dt * 128:(dt + 1) * 128],
                                xf8_sb[j][:, :, t0:t0 + 512],
                                start=(j == 0), stop=(j == NCP - 1),
                                perf_mode=DR)
                    nc.scalar.activation(
                        outT[dt][:, tcnp * 1024:(tcnp + 1) * 1024], ps[:],
                        Ident, bias=b_sb[dt][:])

                def emit_v_group(sip):
                    """vv[2*sip], vv[2*sip+1] via 16 bf16 matmuls + 1 DVE."""
                    ps = pa.tile([128, 1024], F32, name="pa", tag="pa")
                    for c in range(NCC):
                        for hh in range(2):
                            si = 2 * sip + hh
                            nc.tensor.matmul(
                                ps[:, hh * 512:(hh + 1) * 512],
                                xts[c][:, si * 128:(si + 1) * 128],
                                wv_sb[c][:],
                                start=(c == 0), stop=(c == NCC - 1))
                    p4 = ps[:].rearrange("p (k h e) -> p k h e", k=2, e=DK)
                    b4 = bv_sb[:].rearrange("p (k h e) -> p k h e", k=2, e=DK)
                    with nc.allow_low_precision(reason="bf16 V tiles"):
                        nc.vector.tensor_add(
                            v4[:, 2 * sip:2 * sip + 2, :, 0:DK], p4, b4)

                def emit_proj(tt):
                    """y[tt*128 : +128, :] out-projection group."""
                    py = pa.tile([128, 1024], F32, name="pa", tag="pa")
                    for ci in range(NDT):
                        for hh in range(2):
                            nc.tensor.matmul(
                                py[:, hh * 512:(hh + 1) * 512],
                                ctx[ci][:, tt * 128:(tt + 1) * 128],
                                wo_sb[ci][:, hh * 512:(hh + 1) * 512],
                                start=(ci == 0), stop=(ci == NDT - 1))
                    yt = sb.tile([128, D], BF16, name="y_t", tag="y_t", bufs=2)
                    with nc.allow_low_precision(reason="bf16 y output"):
                        nc.vector.tensor_add(yt[:], py[:], bo_sb[:])
                    nc.sync.dma_start(y[tt * 128:(tt + 1) * 128, :], yt[:])

                # ---------------- attention block machinery ----------------
                def make_norm(ti, tbs, pcA, pcB):
                    st = {}

                    def c1():  # ACT: pull denominator rows into [1,1024]
                        st['ds'] = sb.tile([1, 1024], F32, name="ds", tag="ds",
                                           bufs=1)
                        nc.scalar.activation(st['ds'][:, 0:512],
                                             pcA[64:65, :], Ident)

                    def c2():
                        nc.scalar.activation(st['ds'][:, 512:1024],
                                             pcB[64:65, :], Ident)

                    def m1():  # bit-trick reciprocal seed
                        st['rci'] = sb.tile([1, 1024], I32, name="rci",
                                            tag="rci", bufs=1)
                        nc.vector.tensor_scalar(
                            st['rci'][:], st['ds'][:].bitcast(I32),
                            -1.0, C_RECIP, AT.mult, AT.add)

                    def m2():
                        st['aa'] = sb.tile([1, 1024], F32, name="ra", tag="ra",
                                           bufs=1)
                        nc.vector.tensor_mul(
                            st['aa'][:], st['ds'][:],
                            st['rci'][:].bitcast(F32))

                    def m3():
                        st['cc'] = sb.tile([1, 1024], F32, name="rb2",
                                           tag="rb2", bufs=1)
                        nc.vector.tensor_mul(
                            st['cc'][:], st['aa'][:],
                            st['rci'][:].bitcast(F32))

                    def m4():  # r1 = NR*r0 - d*r0^2  (one Newton step)
                        st['r1'] = sb.tile([1, 1024], F32, name="r1", tag="r1",
                                           bufs=1)
                        nc.vector.scalar_tensor_tensor(
                            st['r1'][:], st['rci'][:].bitcast(F32),
                            NR_SCALAR, st['cc'][:], AT.mult, AT.subtract)

                    def b1():  # gpsimd broadcasts to 64 partitions
                        st['rbA'] = sb.tile([64, 512], F32, name="rbA",
                                            tag="rbA", bufs=1)
                        nc.gpsimd.partition_broadcast(st['rbA'][:],
                                                      st['r1'][:, 0:512])

                    def b2():
                        st['rbB'] = sb.tile([64, 512], F32, name="rbB",
                                            tag="rbB", bufs=1)
                        nc.gpsimd.partition_broadcast(st['rbB'][:],
                                                      st['r1'][:, 512:1024])

                    def n7():
                        with nc.allow_low_precision(reason="bf16 ctx"):
                            nc.vector.tensor_mul(ctx[ti][0:64, tbs],
                                                 pcA[0:64, :], st['rbA'][:])

                    def n8():
                        with nc.allow_low_precision(reason="bf16 ctx"):
                            nc.vector.tensor_mul(ctx[ti][64:128, tbs],
                                                 pcB[0:64, :], st['rbB'][:])

                    return [c1, c2, m1, m2, m3, m4, b1, b2, n7, n8]

                pending_norm = []
                slack = []

                def pop_slack():
                    if pending_norm:
                        pending_norm.pop(0)()
                    elif slack:
                        slack.pop(0)()

                # prologue: first qk groups for block (0,0)
                emit_qk_group(wk_sb, bk_sb, kT, 0, 0)
                emit_qk_group(wq_sb, bq_sb, qT, 0, 0)

                for blk in range(NDT * NTB):
                    ti, tb = divmod(blk, NTB)
                    tbs = slice(tb * 512, (tb + 1) * 512)
                    h0d = (2 * ti) * VW
                    h1d = h0d + VW
                    pcA = pc.tile([65, 512], F32, name="pcA", tag="pc")
                    pcB = pc.tile([65, 512], F32, name="pcB", tag="pc")

                    if blk == 0:
                        blk_tasks = [lambda s=s: emit_v_group(s)
                                     for s in range(5)]
                        blk_tasks.insert(3, lambda: emit_qk_group(
                            wk_sb, bk_sb, kT, 0, 1))
                        blk_tasks += [lambda s=s: emit_v_group(s)
                                      for s in range(5, 8)]
                    else:
                        blk_tasks = []
                        if tb == 1 and ti < NDT:
                            blk_tasks.append(lambda t=ti: emit_qk_group(
                                wq_sb, bq_sb, qT, t, 1))
                            if ti + 1 < NDT:
                                blk_tasks.append(lambda t=ti + 1: emit_qk_group(
                                    wk_sb, bk_sb, kT, t, 0))
                        elif tb == 2 and ti + 1 < NDT:
                            blk_tasks.append(lambda t=ti + 1: emit_qk_group(
                                wk_sb, bk_sb, kT, t, 1))
                            blk_tasks.append(lambda t=ti + 1: emit_qk_group(
                                wq_sb, bq_sb, qT, t, 0))
                        if ti == NDT - 1 and tb > 0:
                            for tt in range((tb - 1) * 4, tb * 4):
                                blk_tasks.append(lambda t=tt: emit_proj(t))
                    slack.extend(blk_tasks)

                    def emit_scores_exp(si):
                        ss = slice(si * 128, (si + 1) * 128)
                        sp = pa.tile([128, 1024], F32, name="pa", tag="pa")
                        for half in range(2):
                            nc.tensor.matmul(
                                sp[:, half * 512:(half + 1) * 512],
                                kT[ti][half * 64:half * 64 + 64, ss],
                                qT[ti][half * 64:half * 64 + 64, tbs],
                                start=True, stop=True)
                        if si in ACT_SI:
                            et = sb.tile([128, 1024], BF16, name="etA",
                                         tag="etA", bufs=4)
                            nc.scalar.activation(et[:], sp[:], Exp,
                                                 scale=SCALE,
                                                 bias=actbias_sb[:])
                            return et[:]
                        et = sb.tile([128, 1024], I16, name="etS", tag="etS",
                                     bufs=4)
                        nc.vector.tensor_scalar(et[:], sp[:], A16, B16S,
                                                AT.mult, AT.add)
                        return et[:].bitcast(BF16)

                    def emit_pv(si, ev):
                        nc.tensor.matmul(pcA[:], vv[:, si * HPG * VW + h0d:
                                                    si * HPG * VW + h0d + VW],
                                         ev[:, 0:512],
                                         start=(si == 0), stop=(si == NSI - 1))
                        nc.tensor.matmul(pcB[:], vv[:, si * HPG * VW + h1d:
                                                    si * HPG * VW + h1d + VW],
                                         ev[:, 512:1024],
                                         start=(si == 0), stop=(si == NSI - 1))

                    # 2-si slots: [S,S,S,S][exp,exp][PV,PV,PV,PV] batches
                    pend = []
                    for sp in range(NSI // 2):
                        pend.append((2 * sp, emit_scores_exp(2 * sp)))
                        pend.append((2 * sp + 1, emit_scores_exp(2 * sp + 1)))
                        while len(pend) > 2:
                            emit_pv(*pend.pop(0))
                        pop_slack()
                        pop_slack()
                    for item in pend:
                        emit_pv(*item)
                    pending_norm = make_norm(ti, tbs, pcA, pcB)

                # epilogue: overlap last norm with partial out-projections
                # (ci 0-2 don't need the final ctx[3] block)
                part = {}
                for tt in (12, 13):
                    py = pa.tile([128, 1024], F32, name="pa", tag="pa")
                    for ci in range(NDT - 1):
                        for hh in range(2):
                            nc.tensor.matmul(
                                py[:, hh * 512:(hh + 1) * 512],
                                ctx[ci][:, tt * 128:(tt + 1) * 128],
                                wo_sb[ci][:, hh * 512:(hh + 1) * 512],
                                start=(ci == 0), stop=False)
                    part[tt] = py
                while pending_norm:
                    pending_norm.pop(0)()
                while slack:
                    slack.pop(0)()
                for tt in (12, 13):
                    py = part[tt]
                    for hh in range(2):
                        nc.tensor.matmul(
                            py[:, hh * 512:(hh + 1) * 512],
                            ctx[NDT - 1][:, tt * 128:(tt + 1) * 128],
                            wo_sb[NDT - 1][:, hh * 512:(hh + 1) * 512],
                            start=False, stop=True)
                    yt = sb.tile([128, D], BF16, name="y_t", tag="y_t", bufs=2)
                    with nc.allow_low_precision(reason="bf16 y output"):
                        nc.vector.tensor_add(yt[:], py[:], bo_sb[:])
                    nc.sync.dma_start(y[tt * 128:(tt + 1) * 128, :], yt[:])
                for tt in (14, 15):
                    emit_proj(tt)

    nc.compile()
    return nc


_PROGRAM = None


def _get_program():
    global _PROGRAM
    if _PROGRAM is None:
        _PROGRAM = build_program()
    return _PROGRAM


def make_in_maps(x, w_qkv, b_qkv, w_out, b_out):
    x = np.asarray(x, dtype=np.float32)
    w_qkv = np.asarray(w_qkv, dtype=np.float32)
    b_qkv = np.asarray(b_qkv, dtype=np.float32)
    w_out = np.asarray(w_out, dtype=np.float32)
    b_out = np.asarray(b_out, dtype=np.float32)
    tobf = lambda a: np.ascontiguousarray(a).astype(ml_dtypes.bfloat16)

    def tof8_pairs(a, flat=True):
        # (D, M) -> [128, NCP, 2, M] fp8 with d = (2j+k)*128 + p pair layout
        Dd, M = a.shape
        r = a.reshape(NCP, 2, 128, M).transpose(2, 0, 1, 3)
        if flat:
            r = r.reshape(128, -1)
        return np.ascontiguousarray(r).astype(ml_dtypes.float8_e4m3fn)

    in_maps = []
    for core in range(NCORES):
        b, g = divmod(core, G)
        gs = slice(g * HD, (g + 1) * HD)
        bo_part = b_out if g == 0 else np.zeros_like(b_out)
        xTb = x[b].T  # (D, T)
        bvg = b_qkv[2 * D:3 * D][gs]
        in_maps.append({
            "xt": tobf(xTb),
            "xf8": tof8_pairs(xTb, flat=False),
            "wqf8": tof8_pairs(w_qkv[:, 0 * D:1 * D][:, gs]),
            "wkf8": tof8_pairs(w_qkv[:, 1 * D:2 * D][:, gs]),
            "wv": tobf(w_qkv[:, 2 * D:3 * D][:, gs]),
            "bq": np.ascontiguousarray(b_qkv[0 * D:1 * D][gs].reshape(HD, 1)),
            "bk": np.ascontiguousarray(b_qkv[1 * D:2 * D][gs].reshape(HD, 1)),
            "bv2": np.ascontiguousarray(np.broadcast_to(
                np.concatenate([bvg, bvg]), (128, 2 * HD)).astype(np.float32)),
            "wo": tobf(w_out[gs, :]),
            "bo": np.ascontiguousarray(
                np.broadcast_to(bo_part, (128, D)).astype(np.float32)),
        })
    return in_maps


def run(inputs, trace=False, tmpdir=None):
    nc = _get_program()
    in_maps = make_in_maps(**inputs)
    res = run_bass_kernel_spmd(nc, in_maps, list(range(NCORES)),
                               trace=trace, tmpdir=tmpdir)
    parts = [np.asarray(res.results[c]["y"]).astype(np.float32)
             for c in range(NCORES)]
    out = np.empty((B, T, D), dtype=np.float32)
    for b in range(B):
        out[b] = parts[b * G + 0] + parts[b * G + 1]
    return out, res


def kernel(**inputs) -> np.ndarray:
    out, _ = run(inputs, trace=False)
    return out


# revision 12
# speedup vs baseline: 1.0908x; 1.0908x over previous
"""Multi-head self-attention (B=4, T=2048, D=1024, H=16) on 8 trn2 cores — v2.

Sharding: core = b * 2 + g (b = batch, g = head-group of 8 heads).
All PE operands bf16 (f32 PSUM accumulation). Per core:
  Phase 1: Q^T,K^T [d, T] via stationary-weight matmuls (LDW amortized over
           4 t-chunks); V [t, d] tiles via stationary-x chunks.
  Phase 2: per (head-pair ti, t-block tb): 16 s-iters of
           - scores: 2 row-tiled (K=64) concurrent matmuls -> [128s, 512t] x2
           - exp: ACT exact (exp with scale+bias) or DVE 1-pass i16
             Schraudolph (bitcast bf16), per-si schedule; all at a common
             2^-63/c2 output scale (cancels in softmax).
           - PV: 2 col-tiled concurrent matmuls (M=64 each) accumulating a
             head-pair ctx [128, 512] in one PSUM bank (start-once).
           - denominators: 2 col-tiled M=1 ones-matmuls into a shared bank
             at parity positions (0,32)/(64,96), start-once.
           Normalization: bit-trick reciprocal seed + 2 Newton iterations on
           DVE, gpsimd partition_broadcast, DVE multiply -> ctx bf16.
  Phase 3: out-projection from stationary ctx chunks (2 matmuls per LDW),
           DVE bias add, DMA out. Host sums the two head-group partials.
"""

import numpy as np
import ml_dtypes
import concourse.bass as bass
import concourse.bacc as bacc
import concourse.mybir as mybir
import concourse.tile as tile
from concourse.bass_utils import run_bass_kernel_spmd

B, T, D = 4, 2048, 1024
H, DK = 16, 64
G = 2
HPG = H // G          # 8 heads per core
HD = HPG * DK         # 512
NCORES = B * G
SCALE = 1.0 / float(np.sqrt(DK))

F32 = mybir.dt.float32
F8 = mybir.dt.float8e4
BF16 = mybir.dt.bfloat16
I32 = mybir.dt.int32
I16 = mybir.dt.int16
AT = mybir.AluOpType
DR = mybir.MatmulPerfMode.DoubleRow
NCP = 4
Ident = mybir.ActivationFunctionType.Identity
Exp = mybir.ActivationFunctionType.Exp

NCC = D // 128        # 8 contraction chunks
NDT = HD // 128       # 4 head-pair tiles
NTT = T // 128        # 16 t-tiles
NSI = T // 128        # 16 s-tiles
NTB = T // 512        # 4 t-blocks

# ---- exp constants (common output scale 2^-63/c2, cancels in softmax) ----
LOG2E = float(np.log2(np.e))
_ws = np.linspace(1, 2, 4001)
_c2, _c1, _c0 = np.polyfit(_ws, 2 ** (_ws - 1) / _ws, 2)
A16 = float((2 ** 7) * LOG2E * SCALE)
B16S = float((127 - 63) * 2 ** 7 + (2 ** 7) * np.log2(1.0 / _c2) - 7.25)
ACT_BIAS = float(-63 * np.log(2) - np.log(_c2))
C_RECIP = 2129850000.0
NR_SCALAR = 2.0 * (1.0 + 0.00066)
# si with both exp halves on ACT (rest split ACT/DVE by half parity)
ACT_BOTH = {1, 4, 7, 9, 12, 15}


def build_program():
    nc = bacc.Bacc("TRN2", target_bir_lowering=False, debug=False)

    xt = nc.dram_tensor("xt", [D, T], BF16, kind="ExternalInput").ap()
    xf8 = nc.dram_tensor("xf8", [128, NCP, 2, T], F8,
                         kind="ExternalInput").ap()
    wq = nc.dram_tensor("wq", [128, NCP * 2 * HD], F8,
                        kind="ExternalInput").ap()
    wk = nc.dram_tensor("wk", [128, NCP * 2 * HD], F8,
                        kind="ExternalInput").ap()
    wv = nc.dram_tensor("wv", [D, HD], BF16, kind="ExternalInput").ap()
    bq = nc.dram_tensor("bq", [HD, 1], F32, kind="ExternalInput").ap()
    bk = nc.dram_tensor("bk", [HD, 1], F32, kind="ExternalInput").ap()
    bv = nc.dram_tensor("bv", [128, HD], F32, kind="ExternalInput").ap()
    wo = nc.dram_tensor("wo", [HD, D], BF16, kind="ExternalInput").ap()
    bo = nc.dram_tensor("bo", [128, D], F32, kind="ExternalInput").ap()
    y = nc.dram_tensor("y", [T, D], BF16, kind="ExternalOutput").ap()

    with tile.TileContext(nc) as tc:
        with tc.tile_pool(name="persist", bufs=1) as pp:
            qT = [pp.tile([128, T], BF16, name=f"qT{i}", tag=f"qT{i}")
                  for i in range(NDT)]
            kT = [pp.tile([128, T], BF16, name=f"kT{i}", tag=f"kT{i}")
                  for i in range(NDT)]
            VW = HPG * (DK + 1)   # 520
            vv = [pp.tile([128, VW], BF16, name=f"v{i}", tag=f"v{i}")
                  for i in range(NSI)]
            ctx = [pp.tile([128, T], BF16, name=f"ctx{i}", tag=f"ctx{i}")
                   for i in range(NDT)]
            ones_sb = pp.tile([128, 1], BF16, name="ones", tag="ones")
            nc.vector.memset(ones_sb[:], 1.0)
            actbias_sb = pp.tile([128, 1], F32, name="actbias", tag="actbias")
            nc.vector.memset(actbias_sb[:], ACT_BIAS)
            onesw_sb = pp.tile([128, HPG], BF16, name="onesw", tag="onesw")
            nc.vector.memset(onesw_sb[:], 1.0)

            # ============ Phase 1: QKV projections ============
            with tc.tile_pool(name="p1", bufs=1) as p1:
                xts = [p1.tile([128, T], BF16, name=f"xt{c}", tag=f"xt{c}")
                       for c in range(NCC)]
                xf8_sb = [p1.tile([128, 2, T], F8, name=f"xf8_{j}",
                                  tag=f"xf8_{j}") for j in range(NCP)]
                wq_sb = [p1.tile([128, 2 * HD], F8, name=f"wq{j}",
                                 tag=f"wq{j}") for j in range(NCP)]
                wk_sb = [p1.tile([128, 2 * HD], F8, name=f"wk{j}",
                                 tag=f"wk{j}") for j in range(NCP)]
                wv_sb = [p1.tile([128, HD], BF16, name=f"wv{c}", tag=f"wv{c}")
                         for c in range(NCC)]
                for j in range(NCP):
                    nc.scalar.dma_start(wk_sb[j][:],
                                        wk[:, j * 2 * HD:(j + 1) * 2 * HD])
                    nc.scalar.dma_start(wq_sb[j][:],
                                        wq[:, j * 2 * HD:(j + 1) * 2 * HD])
                for j in range(NCP):
                    nc.sync.dma_start(xf8_sb[j][:, :, 0:1024],
                                      xf8[:, j, :, 0:1024])
                for j in range(NCP):
                    nc.sync.dma_start(xf8_sb[j][:, :, 1024:2048],
                                      xf8[:, j, :, 1024:2048])
                for c in range(NCC):
                    eng = nc.sync if c % 2 == 0 else nc.scalar
                    eng.dma_start(xts[c][:], xt[c * 128:(c + 1) * 128, :])
                    eng2 = nc.scalar if c % 2 == 0 else nc.sync
                    eng2.dma_start(wv_sb[c][:], wv[c * 128:(c + 1) * 128, :])
                bq_sb = [p1.tile([128, 1], F32, name=f"bq{i}", tag=f"bq{i}")
                         for i in range(NDT)]
                bk_sb = [p1.tile([128, 1], F32, name=f"bk{i}", tag=f"bk{i}")
                         for i in range(NDT)]
                for i in range(NDT):
                    nc.sync.dma_start(bq_sb[i][:], bq[i * 128:(i + 1) * 128, :])
                    nc.sync.dma_start(bk_sb[i][:], bk[i * 128:(i + 1) * 128, :])
                bv_sb = p1.tile([128, HD], F32, name="bv_sb", tag="bv_sb")
                nc.sync.dma_start(bv_sb[:], bv[:])

                # Q^T / K^T: fp8 DoubleRow, stationary w pair-chunk
                # reused across 4 t-chunks
                with tc.tile_pool(name="p1ps", bufs=2, space="PSUM") as p1ps:
                    for w_sb, b_sb, outT in ((wk_sb, bk_sb, kT),
                                             (wq_sb, bq_sb, qT)):
                        for dt in range(NDT):
                            ps = p1ps.tile([128, T], F32, name="qk_ps",
                                           tag="qk_ps")
                            for j in range(NCP):
                                w3 = w_sb[j][:].rearrange(
                                    "p (k m) -> p k m", k=2)
                                for tcn in range(4):
                                    nc.tensor.matmul(
                                        ps[:, tcn * 512:(tcn + 1) * 512],
                                        w3[:, :, dt * 128:(dt + 1) * 128],
                                        xf8_sb[j][:, :,
                                                  tcn * 512:(tcn + 1) * 512],
                                        start=(j == 0), stop=(j == NCP - 1),
                                        perf_mode=DR)
                            nc.scalar.activation(outT[dt][:], ps[:], Ident,
                                                 bias=b_sb[dt][:])
                # V tiles
                with tc.tile_pool(name="p1psv", bufs=3, space="PSUM") as p1psv:
                    for si in range(NSI):
                        psv = p1psv.tile([128, HD], F32, name="v_ps",
                                         tag="v_ps")
                        for c in range(NCC):
                            nc.tensor.matmul(
                                psv[:],
                                xts[c][:, si * 128:(si + 1) * 128],
                                wv_sb[c][:],
                                start=(c == 0), stop=(c == NCC - 1))
                        v3 = vv[si][:].rearrange("p (h e) -> p h e", e=DK + 1)
                        with nc.allow_low_precision(reason="bf16 V tiles"):
                            nc.vector.tensor_add(
                                v3[:, :, 0:DK],
                                psv[:].rearrange("p (h e) -> p h e", e=DK),
                                bv_sb[:].rearrange("p (h e) -> p h e", e=DK))
                        nc.vector.tensor_copy(
                            v3[:, :, DK:DK + 1],
                            onesw_sb[:, 0:HPG].rearrange("p (h e) -> p h e",
                                                         e=1))

            # ---- phase-3 resources (emitted inline during last hp) ----
            wo_sb = [pp.tile([128, D], BF16, name=f"wo{c}", tag=f"wo{c}")
                     for c in range(NDT)]
            for c in range(NDT):
                nc.sync.dma_start(wo_sb[c][:], wo[c * 128:(c + 1) * 128, :])
            bo_sb = pp.tile([128, D], F32, name="bo_sb", tag="bo_sb")
            nc.sync.dma_start(bo_sb[:], bo[:])

            # ============ Phase 2: attention ============
            with tc.tile_pool(name="p2", bufs=1) as p2, \
                 tc.tile_pool(name="p2sp", bufs=4, space="PSUM") as p2sp, \
                 tc.tile_pool(name="p2pc", bufs=4, space="PSUM") as p2pc:

                def emit_proj(tt):
                    py = p2sp.tile([128, 512], F32, name="sp", tag="sp")
                    py2 = p2sp.tile([128, 512], F32, name="sp", tag="sp")
                    for ci in range(NDT):
                        nc.tensor.matmul(
                            py[:],
                            ctx[ci][:, tt * 128:(tt + 1) * 128],
                            wo_sb[ci][:, 0:512],
                            start=(ci == 0), stop=(ci == NDT - 1))
                        nc.tensor.matmul(
                            py2[:],
                            ctx[ci][:, tt * 128:(tt + 1) * 128],
                            wo_sb[ci][:, 512:1024],
                            start=(ci == 0), stop=(ci == NDT - 1))
                    yt = p2.tile([128, D], BF16, name="y_t", tag="y_t",
                                 bufs=3)
                    with nc.allow_low_precision(reason="bf16 y output"):
                        nc.vector.tensor_add(yt[:, 0:512], py[:], bo_sb[:, 0:512])
                        nc.vector.tensor_add(yt[:, 512:1024], py2[:],
                                             bo_sb[:, 512:1024])
                    nc.sync.dma_start(y[tt * 128:(tt + 1) * 128, :], yt[:])
                pending_norm = []
                pending_proj = []
                for ti in range(NDT):
                    h0d = (2 * ti) * (DK + 1)      # head col offsets in vv
                    h1d = h0d + (DK + 1)
                    for tb in range(NTB):
                        tbs = slice(tb * 512, (tb + 1) * 512)
                        pcA = p2pc.tile([65, 512], F32, name="pcA", tag="pc")
                        pcB = p2pc.tile([65, 512], F32, name="pcB", tag="pc")

                        def emit_scores_exp(si):
                            ss = slice(si * 128, (si + 1) * 128)
                            evs = []
                            for half in range(2):
                                sp = p2sp.tile([128, 512], F32, name="sp",
                                               tag="sp")
                                nc.tensor.matmul(
                                    sp[:],
                                    kT[ti][half * 64:half * 64 + 64, ss],
                                    qT[ti][half * 64:half * 64 + 64, tbs],
                                    start=True, stop=True)
                                if si in ACT_BOTH or (si + half) % 2 == 0:
                                    et = p2.tile([128, 512], BF16, name="etA",
                                                 tag="etA", bufs=8)
                                    nc.scalar.activation(et[:], sp[:],
                                                         Exp, scale=SCALE,
                                                         bias=actbias_sb[:])
                                    evs.append(et[:])
                                else:
                                    et = p2.tile([128, 512], I16, name="etS",
                                                 tag="etS", bufs=8)
                                    nc.vector.tensor_scalar(et[:], sp[:],
                                                            A16, B16S,
                                                            AT.mult, AT.add)
                                    evs.append(et[:].bitcast(BF16))
                            return evs

                        def emit_pv(si, evs):
                            nc.tensor.matmul(pcA[:],
                                             vv[si][:, h0d:h0d + DK + 1],
                                             evs[0],
                                             start=(si == 0),
                                             stop=(si == NSI - 1))
                            nc.tensor.matmul(pcB[:],
                                             vv[si][:, h1d:h1d + DK + 1],
                                             evs[1],
                                             start=(si == 0),
                                             stop=(si == NSI - 1))

                        pend = []
                        for si in range(NSI):
                            pend.append((si, emit_scores_exp(si)))
                            if len(pend) > 2:
                                emit_pv(*pend.pop(0))
                            if pending_norm and si >= 4 and pending_norm:
                                pending_norm.pop(0)()
                        for item in pend:
                            emit_pv(*item)
                        while pending_proj:
                            emit_proj(pending_proj.pop(0))
                        while pending_norm:
                            pending_norm.pop(0)()

                        # ---- normalization (spread into the next block) ----
                        def make_norm(ti, tbs, pcA, pcB):
                            st = {}

                            def n1():
                                st['dsum'] = pp.tile([1, 1024], F32,
                                                     name="dsum", tag="dsum",
                                                     bufs=2)
                                nc.vector.tensor_copy(st['dsum'][:, 0:512],
                                                      pcA[64:65, :])

                            def n2():
                                nc.vector.tensor_copy(st['dsum'][:, 512:1024],
                                                      pcB[64:65, :])

                            def n3():
                                st['rci'] = pp.tile([1, 1024], I32,
                                                    name="rci", tag="rci",
                                                    bufs=2)
                                nc.vector.tensor_scalar(
                                    st['rci'][:], st['dsum'][:].bitcast(I32),
                                    -1.0, C_RECIP, AT.mult, AT.add)

                            def n4():
                                st['aa'] = pp.tile([1, 1024], F32, name="ra",
                                                   tag="ra", bufs=2)
                                nc.vector.tensor_mul(
                                    st['aa'][:], st['dsum'][:],
                                    st['rci'][:].bitcast(F32))

                            def n5():
                                st['cc'] = pp.tile([1, 1024], F32, name="rb2",
                                                   tag="rb2", bufs=2)
                                nc.vector.tensor_mul(
                                    st['cc'][:], st['aa'][:],
                                    st['rci'][:].bitcast(F32))

                            def n6():
                                st['r1'] = pp.tile([1, 1024], F32, name="r1",
                                                   tag="r1", bufs=2)
                                nc.vector.scalar_tensor_tensor(
                                    st['r1'][:], st['rci'][:].bitcast(F32),
                                    NR_SCALAR, st['cc'][:], AT.mult,
                                    AT.subtract)
                                st['rb'] = pp.tile([64, 1024], F32,
                                                   name="rbb", tag="rbb",
                                                   bufs=2)
                                nc.gpsimd.partition_broadcast(st['rb'][:],
                                                              st['r1'][:])

                            def n7():
                                with nc.allow_low_precision(reason="bf16 ctx"):
                                    nc.vector.tensor_mul(ctx[ti][0:64, tbs],
                                                         pcA[0:64, :],
                                                         st['rb'][:, 0:512])

                            def n8():
                                with nc.allow_low_precision(reason="bf16 ctx"):
                                    nc.vector.tensor_mul(ctx[ti][64:128, tbs],
                                                         pcB[0:64, :],
                                                         st['rb'][:, 512:1024])

                            return [n1, n2, n3, n4, n5, n6, n7, n8]

                        pending_norm = make_norm(ti, tbs, pcA, pcB)
                        if ti == NDT - 1 and tb > 0:
                            for tt in range((tb - 1) * 4, tb * 4):
                                pending_proj.append(tt)
                while pending_proj:
                    emit_proj(pending_proj.pop(0))
                while pending_norm:
                    pending_norm.pop(0)()
                for tt in range(12, 16):
                    emit_proj(tt)

    nc.compile()
    return nc


_PROGRAM = None


def _get_program():
    global _PROGRAM
    if _PROGRAM is None:
        _PROGRAM = build_program()
    return _PROGRAM


def make_in_maps(x, w_qkv, b_qkv, w_out, b_out):
    x = np.asarray(x, dtype=np.float32)
    w_qkv = np.asarray(w_qkv, dtype=np.float32)
    b_qkv = np.asarray(b_qkv, dtype=np.float32)
    w_out = np.asarray(w_out, dtype=np.float32)
    b_out = np.asarray(b_out, dtype=np.float32)
    tobf = lambda a: np.ascontiguousarray(a).astype(ml_dtypes.bfloat16)

    def tof8_pairs(a, flat=True):
        Dd, M = a.shape
        r = a.reshape(NCP, 2, 128, M).transpose(2, 0, 1, 3)
        if flat:
            r = r.reshape(128, -1)
        return np.ascontiguousarray(r).astype(ml_dtypes.float8_e4m3fn)

    in_maps = []
    for core in range(NCORES):
        b, g = divmod(core, G)
        gs = slice(g * HD, (g + 1) * HD)
        bo_part = b_out if g == 0 else np.zeros_like(b_out)
        xTb = x[b].T
        in_maps.append({
            "xt": tobf(xTb),
            "xf8": tof8_pairs(xTb, flat=False),
            "wq": tof8_pairs(w_qkv[:, 0 * D:1 * D][:, gs]),
            "wk": tof8_pairs(w_qkv[:, 1 * D:2 * D][:, gs]),
            "wv": tobf(w_qkv[:, 2 * D:3 * D][:, gs]),
            "bq": np.ascontiguousarray(b_qkv[0 * D:1 * D][gs].reshape(HD, 1)),
            "bk": np.ascontiguousarray(b_qkv[1 * D:2 * D][gs].reshape(HD, 1)),
            "bv": np.ascontiguousarray(
                np.broadcast_to(b_qkv[2 * D:3 * D][gs], (128, HD)).astype(np.float32)),
            "wo": tobf(w_out[gs, :]),
            "bo": np.ascontiguousarray(
                np.broadcast_to(bo_part, (128, D)).astype(np.float32)),
        })
    return in_maps


def run(inputs, trace=False, tmpdir=None):
    nc = _get_program()
    in_maps = make_in_maps(**inputs)
    res = run_bass_kernel_spmd(nc, in_maps, list(range(NCORES)),
                               trace=trace, tmpdir=tmpdir)
    parts = [np.asarray(res.results[c]["y"]).astype(np.float32)
             for c in range(NCORES)]
    out = np.empty((B, T, D), dtype=np.float32)
    for b in range(B):
        out[b] = parts[b * G + 0] + parts[b * G + 1]
    return out, res


def kernel(**inputs) -> np.ndarray:
    out, _ = run(inputs, trace=False)
    return out

